# revision 36
# baseline (speedup 1.0000x reference)
"""MinGRU (parallel log-space scan) Trainium2 Bass kernel.

Problem (hardcoded):
    x:    [B=8, S=4096, D=1024] f32
    W_hg: [D=1024, 2*D=2048]    f32
    out:  [B=8, S=4096, D=1024] f32

    hg = x @ W_hg ; hidden, gate = split(hg)
    h_t = (1-z_t) * h_{t-1} + z_t * g(hidden_t),  z = sigmoid(gate),
    g(v) = v + 0.5 if v >= 0 else sigmoid(v)  ==  max(v + 0.5, sigmoid(v))

Sharding: data-parallel over batch, one batch row per NeuronCore (8 cores),
W_hg replicated.

Layout strategy: the scan must run along the free dimension (channels on
partitions), so the device works entirely in the transposed layout
hg^T/h^T = [channels, seq]. The host packs x per batch row into
per-chunk-contiguous bf16 blocks and W into per-k-slice-contiguous bf16
blocks (gate half first within each slice) so every SBUF load is a single
DMA instruction (the Sync engine serializes DMA issues at ~0.6us each).

bf16 matmuls: 1 cyc/row on the PE like fp32r, but FWL (fast weight load)
hides the LDWEIGHTS stream behind the matmuls, and the x/W DMA volume
halves. Accuracy: ~2.3e-3 max rel err, far below the 2e-2 gate.
The matmul stream (1024 x N=512 MMs) runs at ~216ns/MM = the warm-clock
roofline; total exec ~241us of which ~221us is that stream, ~10us is
fixed framework preamble/teardown, and the rest is head DMA + tail
drain.

Approaches evaluated and rejected (measured, not guessed):
  - fp8e4 DoubleRow (2 k-tiles/instr): e4m3 quantization of x/W puts
    final max rel err at 3.5e-2 (gate 2e-2); every hybrid split
    (gate-only fp8, k-fraction fp8) still lands 2.6-3.4e-2. int8 would
    pass at 1.7e-2 but uint8 matmul has no toolchain support.
  - N=1024 moving operands: walrus rejects s3d3_mm_num_elements --
    f32 PSUM out must fit one 2KB bank, so N<=512 on TRN2.
  - bf16 pointwise: the DVE's 2x packed mode is not implemented for
    stt/scan uops; bf16 runs SLOWER (scan 740->1250ns).
  - Splitting the head DMAs finer (per-j pieces, Scalar-DGE issues):
    transfers under ~512KB are ring-latency-bound (~2 packets/ring on
    16 rings), so each split ADDS ~0.7us; x0-whole-first is optimal.
  - Smaller last chunks ([512x7,256,256]) and merged/finer tail
    pieces: all lose to DVE per-instruction overhead + sem hops.

Per-core pipeline over seq chunks of C=512:
  one DMA for the x^T chunk block [128, 8j x C] (bf16)
  -> per k: bf16 matmuls gate then hidden, accumulated in PSUM
     (a = sigmoid(-gate) on ACT overlaps the hidden matmuls)
  -> DVE: gh = (hidden + 0.5) max sigh ; bneg = (a - 1) * gh
  -> DVE: h = scan(a * h_prev) - bneg   (carry chained across chunks)
  -> DMA h^T tile straight to DRAM out^T.

Head: the DMA fabric is critical-BYTE bound at ~360 GB/s with ring FIFO
completion in issue order, so the order is: chunk 0's x whole (1MB, the
gating transfer), then W k-slice 0 as two halves -- the gate half
completes ~0.5us before the hidden half and the first (gate) matmul
group starts on it. PE p-state warmup matmuls on a memset tile bridge
the ~5us DMA wait so the 0.65->2.4 GHz clock ramp overlaps the head
(13 warmups; more delays the real stream, fewer leaves idle gaps).
A 4-byte fence DMA reading x0's tail holds the non-critical loads
(w1..w7, x1) back until chunk 0 has landed so they don't steal
bandwidth from it.

Tail: the last two k-tiles' hidden accumulation is split in half and
the pointwise/scan/store runs in halves so the final stores overlap
the final scans (k6 split too: its DVE work otherwise backlogs into
k7's matmul window).
"""

import numpy as np

import concourse.bacc as bacc
import concourse.tile as tile
from concourse import mybir

B, S, D = 8, 4096, 1024
N_CORES = 8
P = 128  # partitions
# Seq chunk schedule: uniform 512 (the PSUM-bank maximum). Smaller lead-in
# chunks were tried and lose: the extra matmul instructions and pipeline
# gaps cost more than the smaller critical head DMA saves.
CHUNKS = [512] * 8
CHUNK_OFF = [sum(CHUNKS[:i]) for i in range(len(CHUNKS))]
assert sum(CHUNKS) == S
N_DT = D // P  # 8 d-tiles (contraction)
N_KT = D // P  # 8 output channel tiles (hidden dim = D)
# packed w k-slice: gate j0-5 (the j6-7 pair lives in the fp8 tensor),
# then hidden j0-7
GCOLS = 6 * P
HCOLS = N_DT * P
WBLK = GCOLS + HCOLS

F32 = mybir.dt.float32
BF16 = mybir.dt.bfloat16
FP8 = mybir.dt.float8e4
MM_DT = BF16

_COMPILED = {}


def _build():
    nc = bacc.Bacc(
        "TRN2", target_bir_lowering=False, debug=False, num_devices=N_CORES
    )
    # packed layouts (see make_in_maps): one contiguous run per SBUF load
    xt_d = nc.dram_tensor(
        "xt", [P, N_DT * S], MM_DT, kind="ExternalInput"
    ).ap()
    w_d = nc.dram_tensor(
        "w", [P, N_KT * WBLK], MM_DT, kind="ExternalInput"
    ).ap()
    out_d = nc.dram_tensor("outT", [D, S], F32, kind="ExternalOutput").ap()
    # fp8 side tensors for the gate's j6-j7 DoubleRow pair:
    # xt8 per-chunk blocks [p, (j6,j7), t]; w8 gate slices [p, k, (j6,j7), m]
    xt8_d = nc.dram_tensor("xt8", [P, 2 * S], FP8, kind="ExternalInput").ap()
    w8_d = nc.dram_tensor(
        "w8", [P, N_KT * 2 * P], FP8, kind="ExternalInput"
    ).ap()

    AL = mybir.AluOpType
    SIG = mybir.ActivationFunctionType.Sigmoid

    with tile.TileContext(nc) as tc:
        with (
            tc.tile_pool(name="wpool", bufs=1) as wpool,
            tc.tile_pool(name="xtp", bufs=3) as xt_pool,
            tc.tile_pool(name="pw", bufs=3) as pw_pool,
            tc.tile_pool(name="hp", bufs=3) as h_pool,
            tc.tile_pool(name="pshg", bufs=8, space="PSUM") as psum_hg,
        ):
            w_tile = wpool.tile([P, N_KT * WBLK], MM_DT, name="w_tile")

            def wload(k):
                nc.sync.dma_start(
                    w_tile[:, k * WBLK : (k + 1) * WBLK],
                    w_d[:, k * WBLK : (k + 1) * WBLK],
                )

            def load_x_chunk(sc, name):
                csz = CHUNKS[sc]
                off = N_DT * CHUNK_OFF[sc]
                t = xt_pool.tile([P, N_DT * csz], MM_DT, tag="xc", name=name)
                nc.sync.dma_start(t[:], xt_d[:, off : off + N_DT * csz])
                return t

            def load_x8_chunk(sc):
                csz = CHUNKS[sc]
                off = 2 * CHUNK_OFF[sc]
                t = xt_pool.tile([P, 2 * csz], FP8, tag="x8")
                nc.sync.dma_start(t[:], xt8_d[:, off : off + 2 * csz])
                return t

            # PE p-state warmup: the tensor engine ramps 0.65->2.4 GHz over
            # ~3us of continuous execution. Run garbage matmuls on a
            # memset tile (PSUM never read) while the first real DMAs are
            # in flight so the ramp cost overlaps the head instead of the
            # real stream. Two memsets on different engines so the first
            # LDWEIGHTS only waits for the small Vector one.
            warm = xt_pool.tile([P, 512], MM_DT, tag="warm", bufs=1)
            nc.vector.memset(warm[:, 0:P], 0.0)
            nc.gpsimd.memset(warm[:, P:512], 0.0)
            warm_ps = psum_hg.tile([P, 512], F32, tag="ph")
            for i in range(13):
                nc.tensor.matmul(
                    warm_ps[:], warm[:, 0:P], warm[:],
                    start=(i == 0), stop=(i == 12),
                )

            # Critical path first: chunk 0 of x^T whole (the largest
            # transfer; any bytes ahead of it in the ring FIFO delay it
            # and splitting pays ~0.65us per extra DMA), then the k0 w
            # slice in two halves -- the gate half completes first so
            # the first matmul group starts ~0.5us before the hidden
            # half lands.
            CS0 = CHUNKS[0]
            w8_tile = wpool.tile([P, N_KT * 2 * P], FP8, name="w8_tile")
            x0 = load_x_chunk(0, "x0")
            # the first real matmul is k0's gate DR (fp8): its operands
            # (x8 chunk 0 + w8's k0 slice, 160KB) ride right behind x0,
            # ahead of the bf16 w halves; the rest of w8 comes after the
            # fence (k1's DR needs it ~3us later)
            x8_0 = load_x8_chunk(0)
            nc.sync.dma_start(w8_tile[:, 0 : 2 * P], w8_d[:, 0 : 2 * P])
            nc.sync.dma_start(w_tile[:, 0:GCOLS], w_d[:, 0:GCOLS])
            nc.sync.dma_start(w_tile[:, GCOLS:WBLK], w_d[:, GCOLS:WBLK])
            # Hold back the non-critical loads until x0 has landed so they
            # don't steal DMA bandwidth from it: this 4-byte DMA reads x0,
            # so the in-order Sync engine blocks here until x0 completes.
            fence = xt_pool.tile([P, 2], MM_DT, tag="fence", bufs=1)
            nc.sync.dma_start(
                fence[0:1, 0:2], x0[0:1, N_DT * CS0 - 2 : N_DT * CS0]
            )
            wload(1)
            nc.sync.dma_start(
                w8_tile[:, 2 * P :], w8_d[:, 2 * P :]
            )
            wload(2)
            wload(3)
            x1 = load_x_chunk(1, "x1")
            x8_1 = load_x8_chunk(1)
            for k in range(4, N_KT):
                wload(k)

            import dataclasses as _dc

            def dr_ap(base, blk_stride, n_inner):
                # turn a 2D [128, n_inner] slice into the 3-level
                # [K, 2, n_inner] AP DoubleRow expects (two contraction
                # tiles per instruction)
                return _dc.replace(
                    base,
                    ap=mybir.VecI64Pair(
                        [list(base.ap[0]), [blk_stride, 2], [1, n_inner]]
                    ),
                )

            # lhsT slices: w_sb[kk][j]; kk in [0,8) hidden (j0-7),
            # [8,16) gate (j0-5 only; j6-7 are the fp8 DR pair)
            w_sb = [
                [
                    w_tile[
                        :,
                        k * WBLK + (GCOLS if b == 0 else 0) + j * P :
                        k * WBLK + (GCOLS if b == 0 else 0) + (j + 1) * P,
                    ]
                    for j in range(HCOLS // P if b == 0 else GCOLS // P)
                ]
                for b in range(2)
                for k in range(N_KT)
            ]

            prev_h = [None] * N_KT
            for sc, csz in enumerate(CHUNKS):
                s0 = CHUNK_OFF[sc]
                last_chunk = sc == len(CHUNKS) - 1
                if sc == 0:
                    xts, x8s = x0, x8_0
                elif sc == 1:
                    xts, x8s = x1, x8_1
                else:
                    xts = load_x_chunk(sc, None)
                    x8s = load_x8_chunk(sc)

                def mm_group(ps, kk, lo, hi):
                    for j in range(N_DT):
                        nc.tensor.matmul(
                            ps[:],
                            w_sb[kk][j],
                            xts[:, j * csz + lo : j * csz + hi],
                            start=(j == 0),
                            stop=(j == N_DT - 1),
                        )

                for k in range(N_KT):
                    # split the hidden accumulation and pointwise for the
                    # last TWO k-tiles: k6's DVE work otherwise backlogs
                    # into k7's matmul window and extends the tail drain.
                    # (Finer pieces than halves lose: the DVE's ~250ns
                    # per-instruction overhead bloats the serial tail
                    # chain more than the smaller last piece saves.)
                    if last_chunk and k >= N_KT - 2:
                        pieces = [csz // 2, csz // 2]
                    else:
                        pieces = [csz]
                    last_k = len(pieces) > 1
                    # gate first: a = sigmoid(-gate) is ready while the
                    # hidden matmuls run, shortening the per-k tail chain
                    pg = psum_hg.tile([P, csz], F32, tag="ph")  # gate
                    # gate j6-j7 as one fp8 DoubleRow matmul (2 k-tiles
                    # per instruction at ~1.4x bf16 throughput); e4m3
                    # noise on a quarter of the gate contraction costs
                    # 1.49e-2 final rel err (gate 2e-2, bf16 2.3e-3)
                    nc.tensor.matmul(
                        pg[:],
                        dr_ap(w8_tile[:, k * 2 * P : k * 2 * P + P], P, P),
                        dr_ap(x8s[:, 0:csz], csz, csz),
                        start=True, stop=False,
                        perf_mode=mybir.MatmulPerfMode.DoubleRow,
                        skip_group_check=True,
                    )
                    for j in range(6):
                        nc.tensor.matmul(
                            pg[:], w_sb[N_KT + k][j],
                            xts[:, j * csz : (j + 1) * csz],
                            start=False, stop=(j == 5),
                        )
                    a_t = pw_pool.tile([P, csz], F32, tag="a")
                    nc.scalar.activation(a_t[:], pg[:], SIG, scale=-1.0)
                    if last_k:
                        # split accumulation (separate PSUM banks: a start
                        # flag zeroes the whole 2KB zero-region) so the
                        # pointwise tail starts before the final matmul
                        phs = []
                        off = 0
                        for psz in pieces:
                            ph_p = psum_hg.tile([P, psz], F32, tag="ph")
                            for j in range(N_DT):
                                nc.tensor.matmul(
                                    ph_p[:], w_sb[k][j],
                                    xts[:, j * csz + off : j * csz + off + psz],
                                    start=(j == 0), stop=(j == N_DT - 1),
                                )
                            phs.append((off, off + psz, ph_p))
                            off += psz

                        def ph_piece(lo, hi):
                            for (plo, phi, ph_p) in phs:
                                if lo >= plo and hi <= phi:
                                    return ph_p[:, lo - plo : hi - plo]
                            raise AssertionError((lo, hi))
                    else:
                        ph = psum_hg.tile([P, csz], F32, tag="ph")  # hidden
                        mm_group(ph, k, 0, csz)

                        def ph_piece(lo, hi):
                            return ph[:, lo:hi]

                    # pointwise/scan pieces match the hidden PSUM pieces
                    # (finer splits lose to per-instruction overhead since
                    # all three pointwise ops serialize on the DVE)
                    sigh = pw_pool.tile([P, csz], F32, tag="sigh")
                    gh = pw_pool.tile([P, csz], F32, tag="gh")
                    bneg = pw_pool.tile([P, csz], F32, tag="bneg")
                    h = h_pool.tile([P, csz], F32, tag=f"h{k}")
                    lo = 0
                    for psz in pieces:
                        lo, hi = lo, lo + psz
                        php = ph_piece(lo, hi)
                        # sigh = sigmoid(hidden)
                        nc.scalar.activation(sigh[:, lo:hi], php, SIG)
                        # g(hidden) = max(hidden + 0.5, sigmoid(hidden))
                        nc.vector.scalar_tensor_tensor(
                            gh[:, lo:hi], php, 0.5, sigh[:, lo:hi],
                            op0=AL.add, op1=AL.max,
                        )
                        # bneg = (a - 1) * g = -(z * g)
                        nc.vector.scalar_tensor_tensor(
                            bneg[:, lo:hi], a_t[:, lo:hi], 1.0, gh[:, lo:hi],
                            op0=AL.subtract, op1=AL.mult,
                        )
                        # h_t = a_t * h_{t-1} - bneg_t  (linear recurrence)
                        if lo == 0:
                            init = (
                                0.0
                                if prev_h[k] is None
                                else prev_h[k][:, -1:]
                            )
                        else:
                            init = h[:, lo - 1 : lo]
                        nc.vector.tensor_tensor_scan(
                            h[:, lo:hi], a_t[:, lo:hi], bneg[:, lo:hi], init,
                            op0=AL.mult, op1=AL.subtract,
                        )
                        nc.sync.dma_start(
                            out_d[k * P : (k + 1) * P, s0 + lo : s0 + hi],
                            h[:, lo:hi],
                        )
                        lo = hi
                    prev_h[k] = h
    nc.compile()
    return nc


def _get_nc():
    key = str(MM_DT)
    if key not in _COMPILED:
        _COMPILED[key] = _build()
    return _COMPILED[key]


def make_in_maps(x: np.ndarray, W_hg: np.ndarray) -> list[dict]:
    import ml_dtypes

    bf = ml_dtypes.bfloat16
    x = np.asarray(x, dtype=np.float32)
    w = np.asarray(W_hg, dtype=np.float32)

    # x pack: per-chunk contiguous blocks [p, sc][j, t]
    def pack_x(xb):
        blocks = []
        for sc, csz in enumerate(CHUNKS):
            s0 = CHUNK_OFF[sc]
            blk = xb[s0 : s0 + csz, :]  # [csz, D]
            blocks.append(
                blk.reshape(csz, N_DT, P).transpose(2, 1, 0).reshape(P, -1)
            )
        return np.ascontiguousarray(np.concatenate(blocks, axis=1).astype(bf))

    xp = [pack_x(x[b]) for b in range(N_CORES)]
    # w pack per k-slice: gate j0-5 then hidden j0-7 (gate j6-7 live
    # in the fp8 tensor instead)
    g = (
        w[:, D:]
        .reshape(N_DT, P, N_KT, P)[:6]
        .transpose(1, 2, 0, 3)
        .reshape(P, N_KT, GCOLS)
    )
    hh = (
        w[:, :D]
        .reshape(N_DT, P, N_KT, P)
        .transpose(1, 2, 0, 3)
        .reshape(P, N_KT, HCOLS)
    )
    wp = np.ascontiguousarray(
        np.concatenate([g, hh], axis=2).reshape(P, N_KT * WBLK).astype(bf)
    )
    # fp8 packs for the gate j6-j7 DoubleRow pair
    f8 = ml_dtypes.float8_e4m3

    def pack_x8(xb):
        blocks = []
        for sc, csz in enumerate(CHUNKS):
            s0 = CHUNK_OFF[sc]
            blk = xb[s0 : s0 + csz, 6 * P :]  # [csz, 256]
            blocks.append(
                blk.reshape(csz, 2, P).transpose(2, 1, 0).reshape(P, -1)
            )
        return np.ascontiguousarray(
            np.concatenate(blocks, axis=1).astype(f8)
        )

    xp8 = [pack_x8(x[b]) for b in range(N_CORES)]
    # w8: gate columns, d rows 768:1024 -> [p, k, (j6,j7), m]
    wp8 = np.ascontiguousarray(
        w[6 * P :, D:]
        .reshape(2, P, N_KT, P)
        .transpose(1, 2, 0, 3)
        .reshape(P, N_KT * 2 * P)
        .astype(f8)
    )
    return [
        {"xt": xp[b], "w": wp, "xt8": xp8[b], "w8": wp8}
        for b in range(N_CORES)
    ]


def kernel(x: np.ndarray, W_hg: np.ndarray) -> np.ndarray:
    from concourse.bass_utils import run_bass_kernel_spmd

    assert x.shape == (B, S, D) and W_hg.shape == (D, 2 * D)
    nc = _get_nc()
    in_maps = make_in_maps(x, W_hg)
    res = run_bass_kernel_spmd(nc, in_maps, list(range(N_CORES)))
    out = np.empty((B, S, D), dtype=np.float32)
    for b in range(N_CORES):
        out[b] = res.results[b]["outT"].T
    return out


# revision 37
# speedup vs baseline: 1.0153x; 1.0153x over previous
"""MinGRU (parallel log-space scan) Trainium2 Bass kernel.

Problem (hardcoded):
    x:    [B=8, S=4096, D=1024] f32
    W_hg: [D=1024, 2*D=2048]    f32
    out:  [B=8, S=4096, D=1024] f32

    hg = x @ W_hg ; hidden, gate = split(hg)
    h_t = (1-z_t) * h_{t-1} + z_t * g(hidden_t),  z = sigmoid(gate),
    g(v) = v + 0.5 if v >= 0 else sigmoid(v)  ==  max(v + 0.5, sigmoid(v))

Sharding: data-parallel over batch, one batch row per NeuronCore (8 cores),
W_hg replicated.

Layout strategy: the scan must run along the free dimension (channels on
partitions), so the device works entirely in the transposed layout
hg^T/h^T = [channels, seq]. The host packs x per batch row into
per-chunk-contiguous bf16 blocks and W into per-k-slice-contiguous bf16
blocks (gate half first within each slice) so every SBUF load is a single
DMA instruction (the Sync engine serializes DMA issues at ~0.6us each).

bf16 matmuls: 1 cyc/row on the PE like fp32r, but FWL (fast weight load)
hides the LDWEIGHTS stream behind the matmuls, and the x/W DMA volume
halves. Accuracy: ~2.3e-3 max rel err, far below the 2e-2 gate.
The matmul stream (1024 x N=512 MMs) runs at ~216ns/MM = the warm-clock
roofline; total exec ~241us of which ~221us is that stream, ~10us is
fixed framework preamble/teardown, and the rest is head DMA + tail
drain.

Approaches evaluated and rejected (measured, not guessed):
  - fp8e4 DoubleRow (2 k-tiles/instr): e4m3 quantization of x/W puts
    final max rel err at 3.5e-2 (gate 2e-2); every hybrid split
    (gate-only fp8, k-fraction fp8) still lands 2.6-3.4e-2. int8 would
    pass at 1.7e-2 but uint8 matmul has no toolchain support.
  - N=1024 moving operands: walrus rejects s3d3_mm_num_elements --
    f32 PSUM out must fit one 2KB bank, so N<=512 on TRN2.
  - bf16 pointwise: the DVE's 2x packed mode is not implemented for
    stt/scan uops; bf16 runs SLOWER (scan 740->1250ns).
  - Splitting the head DMAs finer (per-j pieces, Scalar-DGE issues):
    transfers under ~512KB are ring-latency-bound (~2 packets/ring on
    16 rings), so each split ADDS ~0.7us; x0-whole-first is optimal.
  - Smaller last chunks ([512x7,256,256]) and merged/finer tail
    pieces: all lose to DVE per-instruction overhead + sem hops.

Per-core pipeline over seq chunks of C=512:
  one DMA for the x^T chunk block [128, 8j x C] (bf16)
  -> per k: bf16 matmuls gate then hidden, accumulated in PSUM
     (a = sigmoid(-gate) on ACT overlaps the hidden matmuls)
  -> DVE: gh = (hidden + 0.5) max sigh ; bneg = (a - 1) * gh
  -> DVE: h = scan(a * h_prev) - bneg   (carry chained across chunks)
  -> DMA h^T tile straight to DRAM out^T.

Head: the DMA fabric is critical-BYTE bound at ~360 GB/s with ring FIFO
completion in issue order, so the order is: chunk 0's x whole (1MB, the
gating transfer), then W k-slice 0 as two halves -- the gate half
completes ~0.5us before the hidden half and the first (gate) matmul
group starts on it. PE p-state warmup matmuls on a memset tile bridge
the ~5us DMA wait so the 0.65->2.4 GHz clock ramp overlaps the head
(13 warmups; more delays the real stream, fewer leaves idle gaps).
A 4-byte fence DMA reading x0's tail holds the non-critical loads
(w1..w7, x1) back until chunk 0 has landed so they don't steal
bandwidth from it.

Tail: the last two k-tiles' hidden accumulation is split in half and
the pointwise/scan/store runs in halves so the final stores overlap
the final scans (k6 split too: its DVE work otherwise backlogs into
k7's matmul window).
"""

import numpy as np

import concourse.bacc as bacc
import concourse.tile as tile
from concourse import mybir

B, S, D = 8, 4096, 1024
N_CORES = 8
P = 128  # partitions
# Seq chunk schedule: uniform 512 (the PSUM-bank maximum). Smaller lead-in
# chunks were tried and lose: the extra matmul instructions and pipeline
# gaps cost more than the smaller critical head DMA saves.
CHUNKS = [512] * 8
CHUNK_OFF = [sum(CHUNKS[:i]) for i in range(len(CHUNKS))]
assert sum(CHUNKS) == S
N_DT = D // P  # 8 d-tiles (contraction)
N_KT = D // P  # 8 output channel tiles (hidden dim = D)
# packed w k-slice: gate j0-5 (the j6-7 pair lives in the fp8 tensor),
# then hidden j0-7
GCOLS = 6 * P
HCOLS = N_DT * P
WBLK = GCOLS + HCOLS

F32 = mybir.dt.float32
BF16 = mybir.dt.bfloat16
FP8 = mybir.dt.float8e4
MM_DT = BF16

_COMPILED = {}


def _build():
    nc = bacc.Bacc(
        "TRN2", target_bir_lowering=False, debug=False, num_devices=N_CORES
    )
    # packed layouts (see make_in_maps): one contiguous run per SBUF load
    xt_d = nc.dram_tensor(
        "xt", [P, N_DT * S], MM_DT, kind="ExternalInput"
    ).ap()
    w_d = nc.dram_tensor(
        "w", [P, N_KT * WBLK], MM_DT, kind="ExternalInput"
    ).ap()
    out_d = nc.dram_tensor("outT", [D, S], F32, kind="ExternalOutput").ap()
    # fp8 side tensors for the gate's j6-j7 DoubleRow pair:
    # xt8 per-chunk blocks [p, (j6,j7), t]; w8 gate slices [p, k, (j6,j7), m]
    xt8_d = nc.dram_tensor("xt8", [P, 2 * S], FP8, kind="ExternalInput").ap()
    w8_d = nc.dram_tensor(
        "w8", [P, N_KT * 2 * P], FP8, kind="ExternalInput"
    ).ap()

    AL = mybir.AluOpType
    SIG = mybir.ActivationFunctionType.Sigmoid

    with tile.TileContext(nc) as tc:
        with (
            tc.tile_pool(name="wpool", bufs=1) as wpool,
            tc.tile_pool(name="xtp", bufs=3) as xt_pool,
            tc.tile_pool(name="pw", bufs=3) as pw_pool,
            tc.tile_pool(name="hp", bufs=3) as h_pool,
            tc.tile_pool(name="pshg", bufs=8, space="PSUM") as psum_hg,
        ):
            w_tile = wpool.tile([P, N_KT * WBLK], MM_DT, name="w_tile")

            def wload(k):
                nc.sync.dma_start(
                    w_tile[:, k * WBLK : (k + 1) * WBLK],
                    w_d[:, k * WBLK : (k + 1) * WBLK],
                )

            def load_x_chunk(sc, name):
                csz = CHUNKS[sc]
                off = N_DT * CHUNK_OFF[sc]
                t = xt_pool.tile([P, N_DT * csz], MM_DT, tag="xc", name=name)
                nc.sync.dma_start(t[:], xt_d[:, off : off + N_DT * csz])
                return t

            # PE p-state warmup: the tensor engine ramps 0.65->2.4 GHz over
            # ~3us of continuous execution. Run garbage matmuls on a
            # memset tile (PSUM never read) while the first real DMAs are
            # in flight so the ramp cost overlaps the head instead of the
            # real stream. Two memsets on different engines so the first
            # LDWEIGHTS only waits for the small Vector one.
            warm = xt_pool.tile([P, 512], MM_DT, tag="warm", bufs=1)
            nc.vector.memset(warm[:, 0:P], 0.0)
            nc.gpsimd.memset(warm[:, P:512], 0.0)
            warm_ps = psum_hg.tile([P, 512], F32, tag="ph")
            for i in range(13):
                nc.tensor.matmul(
                    warm_ps[:], warm[:, 0:P], warm[:],
                    start=(i == 0), stop=(i == 12),
                )

            # Critical path first: chunk 0 of x^T whole (the largest
            # transfer; any bytes ahead of it in the ring FIFO delay it
            # and splitting pays ~0.65us per extra DMA), then the k0 w
            # slice in two halves -- the gate half completes first so
            # the first matmul group starts ~0.5us before the hidden
            # half lands.
            CS0 = CHUNKS[0]
            w8_tile = wpool.tile([P, N_KT * 2 * P], FP8, name="w8_tile")
            x0 = load_x_chunk(0, "x0")
            # the first real matmul is k0's gate DR (fp8): its operands
            # (x8 chunk 0 + w8's k0 slice, 160KB) ride right behind x0,
            # ahead of the bf16 w halves; the rest of w8 and of xt8 come
            # after the fence. xt8 is SBUF-resident as ONE tile loaded by
            # two DMAs -- fewer dma_starts matter because the DMA
            # semaphore pool recycles and each extra issue serializes on
            # an earlier transfer's completion.
            xt8_tile = wpool.tile([P, 2 * S], FP8, name="xt8_tile")
            nc.sync.dma_start(
                xt8_tile[:, 0 : 2 * CS0], xt8_d[:, 0 : 2 * CS0]
            )
            nc.sync.dma_start(w8_tile[:, 0 : 2 * P], w8_d[:, 0 : 2 * P])
            nc.sync.dma_start(w_tile[:, 0:GCOLS], w_d[:, 0:GCOLS])
            nc.sync.dma_start(w_tile[:, GCOLS:WBLK], w_d[:, GCOLS:WBLK])
            # Hold back the non-critical loads until x0 has landed so they
            # don't steal DMA bandwidth from it: this 4-byte DMA reads x0,
            # so the in-order Sync engine blocks here until x0 completes.
            fence = xt_pool.tile([P, 2], MM_DT, tag="fence", bufs=1)
            nc.sync.dma_start(
                fence[0:1, 0:2], x0[0:1, N_DT * CS0 - 2 : N_DT * CS0]
            )
            wload(1)
            nc.sync.dma_start(
                w8_tile[:, 2 * P :], w8_d[:, 2 * P :]
            )
            wload(2)
            wload(3)
            x1 = load_x_chunk(1, "x1")
            nc.sync.dma_start(
                xt8_tile[:, 2 * CS0 :], xt8_d[:, 2 * CS0 :]
            )
            for k in range(4, N_KT):
                wload(k)

            import dataclasses as _dc

            def dr_ap(base, blk_stride, n_inner):
                # turn a 2D [128, n_inner] slice into the 3-level
                # [K, 2, n_inner] AP DoubleRow expects (two contraction
                # tiles per instruction)
                return _dc.replace(
                    base,
                    ap=mybir.VecI64Pair(
                        [list(base.ap[0]), [blk_stride, 2], [1, n_inner]]
                    ),
                )

            # lhsT slices: w_sb[kk][j]; kk in [0,8) hidden (j0-7),
            # [8,16) gate (j0-5 only; j6-7 are the fp8 DR pair)
            w_sb = [
                [
                    w_tile[
                        :,
                        k * WBLK + (GCOLS if b == 0 else 0) + j * P :
                        k * WBLK + (GCOLS if b == 0 else 0) + (j + 1) * P,
                    ]
                    for j in range(HCOLS // P if b == 0 else GCOLS // P)
                ]
                for b in range(2)
                for k in range(N_KT)
            ]

            prev_h = [None] * N_KT
            for sc, csz in enumerate(CHUNKS):
                s0 = CHUNK_OFF[sc]
                last_chunk = sc == len(CHUNKS) - 1
                if sc == 0:
                    xts = x0
                elif sc == 1:
                    xts = x1
                else:
                    xts = load_x_chunk(sc, None)
                x8s = xt8_tile[
                    :, 2 * CHUNK_OFF[sc] : 2 * CHUNK_OFF[sc] + 2 * csz
                ]

                def mm_group(ps, kk, lo, hi):
                    for j in range(N_DT):
                        nc.tensor.matmul(
                            ps[:],
                            w_sb[kk][j],
                            xts[:, j * csz + lo : j * csz + hi],
                            start=(j == 0),
                            stop=(j == N_DT - 1),
                        )

                for k in range(N_KT):
                    # split the hidden accumulation and pointwise for the
                    # last TWO k-tiles: k6's DVE work otherwise backlogs
                    # into k7's matmul window and extends the tail drain.
                    # (Finer pieces than halves lose: the DVE's ~250ns
                    # per-instruction overhead bloats the serial tail
                    # chain more than the smaller last piece saves.)
                    if last_chunk and k >= N_KT - 2:
                        pieces = [csz // 2, csz // 2]
                    else:
                        pieces = [csz]
                    last_k = len(pieces) > 1
                    # gate first: a = sigmoid(-gate) is ready while the
                    # hidden matmuls run, shortening the per-k tail chain
                    pg = psum_hg.tile([P, csz], F32, tag="ph")  # gate
                    # gate j6-j7 as one fp8 DoubleRow matmul (2 k-tiles
                    # per instruction at ~1.4x bf16 throughput); e4m3
                    # noise on a quarter of the gate contraction costs
                    # 1.49e-2 final rel err (gate 2e-2, bf16 2.3e-3)
                    nc.tensor.matmul(
                        pg[:],
                        dr_ap(w8_tile[:, k * 2 * P : k * 2 * P + P], P, P),
                        dr_ap(x8s[:, 0:csz], csz, csz),
                        start=True, stop=False,
                        perf_mode=mybir.MatmulPerfMode.DoubleRow,
                        skip_group_check=True,
                    )
                    for j in range(6):
                        nc.tensor.matmul(
                            pg[:], w_sb[N_KT + k][j],
                            xts[:, j * csz : (j + 1) * csz],
                            start=False, stop=(j == 5),
                        )
                    a_t = pw_pool.tile([P, csz], F32, tag="a")
                    nc.scalar.activation(a_t[:], pg[:], SIG, scale=-1.0)
                    if last_k:
                        # split accumulation (separate PSUM banks: a start
                        # flag zeroes the whole 2KB zero-region) so the
                        # pointwise tail starts before the final matmul
                        phs = []
                        off = 0
                        for psz in pieces:
                            ph_p = psum_hg.tile([P, psz], F32, tag="ph")
                            for j in range(N_DT):
                                nc.tensor.matmul(
                                    ph_p[:], w_sb[k][j],
                                    xts[:, j * csz + off : j * csz + off + psz],
                                    start=(j == 0), stop=(j == N_DT - 1),
                                )
                            phs.append((off, off + psz, ph_p))
                            off += psz

                        def ph_piece(lo, hi):
                            for (plo, phi, ph_p) in phs:
                                if lo >= plo and hi <= phi:
                                    return ph_p[:, lo - plo : hi - plo]
                            raise AssertionError((lo, hi))
                    else:
                        ph = psum_hg.tile([P, csz], F32, tag="ph")  # hidden
                        mm_group(ph, k, 0, csz)

                        def ph_piece(lo, hi):
                            return ph[:, lo:hi]

                    # pointwise/scan pieces match the hidden PSUM pieces
                    # (finer splits lose to per-instruction overhead since
                    # all three pointwise ops serialize on the DVE)
                    sigh = pw_pool.tile([P, csz], F32, tag="sigh")
                    gh = pw_pool.tile([P, csz], F32, tag="gh")
                    bneg = pw_pool.tile([P, csz], F32, tag="bneg")
                    h = h_pool.tile([P, csz], F32, tag=f"h{k}")
                    lo = 0
                    for psz in pieces:
                        lo, hi = lo, lo + psz
                        php = ph_piece(lo, hi)
                        # sigh = sigmoid(hidden)
                        nc.scalar.activation(sigh[:, lo:hi], php, SIG)
                        # g(hidden) = max(hidden + 0.5, sigmoid(hidden))
                        nc.vector.scalar_tensor_tensor(
                            gh[:, lo:hi], php, 0.5, sigh[:, lo:hi],
                            op0=AL.add, op1=AL.max,
                        )
                        # bneg = (a - 1) * g = -(z * g)
                        nc.vector.scalar_tensor_tensor(
                            bneg[:, lo:hi], a_t[:, lo:hi], 1.0, gh[:, lo:hi],
                            op0=AL.subtract, op1=AL.mult,
                        )
                        # h_t = a_t * h_{t-1} - bneg_t  (linear recurrence)
                        if lo == 0:
                            init = (
                                0.0
                                if prev_h[k] is None
                                else prev_h[k][:, -1:]
                            )
                        else:
                            init = h[:, lo - 1 : lo]
                        nc.vector.tensor_tensor_scan(
                            h[:, lo:hi], a_t[:, lo:hi], bneg[:, lo:hi], init,
                            op0=AL.mult, op1=AL.subtract,
                        )
                        nc.sync.dma_start(
                            out_d[k * P : (k + 1) * P, s0 + lo : s0 + hi],
                            h[:, lo:hi],
                        )
                        lo = hi
                    prev_h[k] = h
    nc.compile()
    return nc


def _get_nc():
    key = str(MM_DT)
    if key not in _COMPILED:
        _COMPILED[key] = _build()
    return _COMPILED[key]


def make_in_maps(x: np.ndarray, W_hg: np.ndarray) -> list[dict]:
    import ml_dtypes

    bf = ml_dtypes.bfloat16
    x = np.asarray(x, dtype=np.float32)
    w = np.asarray(W_hg, dtype=np.float32)

    # x pack: per-chunk contiguous blocks [p, sc][j, t]
    def pack_x(xb):
        blocks = []
        for sc, csz in enumerate(CHUNKS):
            s0 = CHUNK_OFF[sc]
            blk = xb[s0 : s0 + csz, :]  # [csz, D]
            blocks.append(
                blk.reshape(csz, N_DT, P).transpose(2, 1, 0).reshape(P, -1)
            )
        return np.ascontiguousarray(np.concatenate(blocks, axis=1).astype(bf))

    xp = [pack_x(x[b]) for b in range(N_CORES)]
    # w pack per k-slice: gate j0-5 then hidden j0-7 (gate j6-7 live
    # in the fp8 tensor instead)
    g = (
        w[:, D:]
        .reshape(N_DT, P, N_KT, P)[:6]
        .transpose(1, 2, 0, 3)
        .reshape(P, N_KT, GCOLS)
    )
    hh = (
        w[:, :D]
        .reshape(N_DT, P, N_KT, P)
        .transpose(1, 2, 0, 3)
        .reshape(P, N_KT, HCOLS)
    )
    wp = np.ascontiguousarray(
        np.concatenate([g, hh], axis=2).reshape(P, N_KT * WBLK).astype(bf)
    )
    # fp8 packs for the gate j6-j7 DoubleRow pair
    f8 = ml_dtypes.float8_e4m3

    def pack_x8(xb):
        blocks = []
        for sc, csz in enumerate(CHUNKS):
            s0 = CHUNK_OFF[sc]
            blk = xb[s0 : s0 + csz, 6 * P :]  # [csz, 256]
            blocks.append(
                blk.reshape(csz, 2, P).transpose(2, 1, 0).reshape(P, -1)
            )
        return np.ascontiguousarray(
            np.concatenate(blocks, axis=1).astype(f8)
        )

    xp8 = [pack_x8(x[b]) for b in range(N_CORES)]
    # w8: gate columns, d rows 768:1024 -> [p, k, (j6,j7), m]
    wp8 = np.ascontiguousarray(
        w[6 * P :, D:]
        .reshape(2, P, N_KT, P)
        .transpose(1, 2, 0, 3)
        .reshape(P, N_KT * 2 * P)
        .astype(f8)
    )
    return [
        {"xt": xp[b], "w": wp, "xt8": xp8[b], "w8": wp8}
        for b in range(N_CORES)
    ]


def kernel(x: np.ndarray, W_hg: np.ndarray) -> np.ndarray:
    from concourse.bass_utils import run_bass_kernel_spmd

    assert x.shape == (B, S, D) and W_hg.shape == (D, 2 * D)
    nc = _get_nc()
    in_maps = make_in_maps(x, W_hg)
    res = run_bass_kernel_spmd(nc, in_maps, list(range(N_CORES)))
    out = np.empty((B, S, D), dtype=np.float32)
    for b in range(N_CORES):
        out[b] = res.results[b]["outT"].T
    return out
